# revision 26
# baseline (speedup 1.0000x reference)
"""5-layer DGL-style GraphConv (AwA2Conv) on 8 Trainium2 NeuronCores.

Math per layer (norm='both'):
    out = D_in^{-1/2} A D_out^{-1/2} (h) @ W + b     (+ leaky_relu except last)

The per-edge weight w_e = dinv_out[src]*dinv_in[dst] is folded into
block-sparse "S" matrices (128-edge x 128-dst chunks) so the sparse
aggregation becomes PE matmuls over dma_gather'ed edge rows. Aggregation
runs at min(Fin, Fout) per layer (matmul commutes with the linear
aggregation). With lhsT = gathered rows the aggregate comes out TRANSPOSED
[F, dst] — exactly the lhsT layout the next dense matmul wants, so the
network runs with zero explicit transposes.

Distribution: dual-block node sharding — core c owns global nodes
[c*3125,(c+1)*3125) u [25000+c*3125, 25000+(c+1)*3125). Each activation
exchange is TWO AllGathers (node halves A/B); gathers for edges with
src<25000 read table A and only wait on the first collective, overlapping
the second. Layer-1 edge rows (gathered from the replicated input x) are
materialized host-side and shipped per core.
"""

import numpy as np
import ml_dtypes

import concourse.bass as bass
import concourse.bacc as bacc
import concourse.mybir as mybir
import concourse.tile as tile
from concourse.bass_utils import run_bass_kernel_spmd

N_NODES = 50000
N_EDGES = 250000
NC = 8
NPC = N_NODES // NC      # 6250 nodes per core
HALF = 25000             # global half boundary (= lo/hi gather split)
HPC = HALF // NC         # 3125 nodes per core per half
P = 128
TPH = 25                 # tiles per half (24x128 + 1x53)
N_TILES = 2 * TPH        # 50 dst tiles per core
DIMS = [300, 1024, 512, 256, 128, 2048]
NEG_SLOPE = 0.01

F32 = mybir.dt.float32
BF16 = mybir.dt.bfloat16
DT = BF16
NPDT = ml_dtypes.bfloat16
I16 = mybir.dt.int16
LRELU = mybir.ActivationFunctionType.Lrelu

LAYER_FA = [300, 512, 256, 128, 128]       # aggregation width
LAYER_FA_PAD = [384, 512, 256, 128, 128]   # gathered row width (256B mult)
GS = 2                                     # tiles per dma_gather call


def _ceil_div(a, b):
    return (a + b - 1) // b


def _tile_start(t):
    return (t // TPH) * HPC + (t % TPH) * P


def _tile_width(t):
    return HPC - (TPH - 1) * P if (t % TPH) == TPH - 1 else P


TILE_STARTS = [_tile_start(t) for t in range(N_TILES)]
TILE_WIDTHS = [_tile_width(t) for t in range(N_TILES)]


# ----------------------------------------------------------------------------
# Host-side graph preprocessing
# ----------------------------------------------------------------------------

def _prep(edge_index, x):
    """Partition edges by (dst core, dst tile), split by src half, pad to
    16-granular per-tile schedules (max across cores -> one SPMD program).

    Returns (sched_lo, sched_hi, per_core). per_core: wrapped int16 gather
    indices, S chunk matrices, and pre-gathered layer-1 x rows.
    """
    GRAN = 128
    src = np.asarray(edge_index[0], dtype=np.int64)
    dst = np.asarray(edge_index[1], dtype=np.int64)
    out_deg = np.bincount(src, minlength=N_NODES).astype(np.float32)
    in_deg = np.bincount(dst, minlength=N_NODES).astype(np.float32)
    dinv_out = 1.0 / np.sqrt(np.maximum(out_deg, 1.0))
    dinv_in = 1.0 / np.sqrt(np.maximum(in_deg, 1.0))
    w = (dinv_out[src] * dinv_in[dst]).astype(np.float32)
    xb = np.asarray(x, dtype=np.float32)

    # dst -> (core, local pos); dual-block sharding
    d_half = dst // HALF
    d_rem = dst % HALF
    d_core = d_rem // HPC
    d_with = d_rem % HPC
    d_pos = d_with + d_half * HPC               # local position in [0, NPC)
    d_tile = d_half * TPH + np.minimum(d_with // P, TPH - 1)
    lo = src < HALF

    key = (d_core * N_TILES + d_tile) * 2 + (~lo).astype(np.int64)
    order = np.lexsort((src, key))
    src_s, w_s, pos_s, key_s = src[order], w[order], d_pos[order], key[order]
    bounds = np.searchsorted(key_s, np.arange(NC * N_TILES * 2 + 1))

    n_lo = np.zeros((NC, N_TILES), dtype=np.int64)
    n_hi = np.zeros((NC, N_TILES), dtype=np.int64)
    for c in range(NC):
        for t in range(N_TILES):
            k = (c * N_TILES + t) * 2
            n_lo[c, t] = bounds[k + 1] - bounds[k]
            n_hi[c, t] = bounds[k + 2] - bounds[k + 1]

    sched_lo = np.maximum(
        np.ceil(n_lo.max(axis=0) / GRAN).astype(np.int64), 1) * GRAN
    sched_hi = np.ceil(n_hi.max(axis=0) / GRAN).astype(np.int64) * GRAN

    per_core = []
    for c in range(NC):
        idx_parts = {True: [], False: []}
        s_parts = {True: [], False: []}
        xg_parts = {True: [], False: []}
        for t in range(N_TILES):
            k = (c * N_TILES + t) * 2
            segs = (
                (True, sched_lo[t], bounds[k], bounds[k + 1]),
                (False, sched_hi[t], bounds[k + 1], bounds[k + 2]),
            )
            for islo, ni, a, b_ in segs:
                ni = int(ni)
                if ni == 0:
                    continue
                n_slots = _ceil_div(ni, P) * P
                ne = b_ - a
                assert ne <= ni
                idx = np.zeros(ni, dtype=np.int64)
                idx[:ne] = src_s[a:b_] - (0 if islo else HALF)
                dstloc = np.full(n_slots, P, dtype=np.int64)
                dstloc[:ne] = pos_s[a:b_] - TILE_STARTS[t]
                wv = np.zeros(n_slots, dtype=np.float32)
                wv[:ne] = w_s[a:b_]
                S = np.zeros((n_slots, P), dtype=np.float32)
                valid = dstloc < P
                S[np.nonzero(valid)[0], dstloc[valid]] = wv[valid]
                s_parts[islo].append(S.reshape(-1, P, P))
                idx_parts[islo].append(idx.reshape(-1, 16).T.astype(np.int16))
                xg = np.zeros((n_slots, 384), dtype=NPDT)
                xg[:ne, :300] = xb[src_s[a:b_]].astype(NPDT)
                xg_parts[islo].append(xg.reshape(-1, P, 384))
        pc = {}
        for islo, nm in ((True, "lo"), (False, "hi")):
            if idx_parts[islo]:
                pc[f"idx_{nm}"] = np.ascontiguousarray(
                    np.tile(np.concatenate(idx_parts[islo], axis=1), (8, 1)))
                # store S / xg partition-major ([P, c, n]) so the group DMA
                # reads each partition contiguously (no strided rearrange)
                pc[f"s_{nm}"] = np.ascontiguousarray(
                    np.concatenate(s_parts[islo], axis=0).transpose(1, 0, 2))
                pc[f"xg_{nm}"] = np.ascontiguousarray(
                    np.concatenate(xg_parts[islo], axis=0).transpose(1, 0, 2))
            else:
                pc[f"idx_{nm}"] = np.zeros((128, 1), dtype=np.int16)
                pc[f"s_{nm}"] = np.zeros((P, 1, P), dtype=np.float32)
                pc[f"xg_{nm}"] = np.zeros((P, 1, 384), dtype=NPDT)
        per_core.append(pc)
    return sched_lo, sched_hi, per_core


# ----------------------------------------------------------------------------
# Bass program builder (depends only on sched_lo / sched_hi)
# ----------------------------------------------------------------------------

def _build(sched_lo, sched_hi):
    nc = bacc.Bacc("TRN2")
    ch_lo = np.ceil(sched_lo / P).astype(np.int64)
    ch_hi = np.ceil(sched_hi / P).astype(np.int64)
    idx_lo_cols = int(sched_lo.sum()) // 16
    idx_hi_cols = max(int(sched_hi.sum()) // 16, 1)
    tot_clo = int(ch_lo.sum())
    tot_chi = max(int(ch_hi.sum()), 1)
    offi_lo = np.concatenate([[0], np.cumsum(sched_lo // 16)]).astype(int)
    offi_hi = np.concatenate([[0], np.cumsum(sched_hi // 16)]).astype(int)
    offc_lo = np.concatenate([[0], np.cumsum(ch_lo)]).astype(int)
    offc_hi = np.concatenate([[0], np.cumsum(ch_hi)]).astype(int)

    xg_lo_d = nc.declare_dram_parameter("xg_lo", [P, tot_clo, 384], DT, isOutput=False)
    xg_hi_d = nc.declare_dram_parameter("xg_hi", [P, tot_chi, 384], DT, isOutput=False)
    Ws, bs = [], []
    for i in range(5):
        fi, fo = DIMS[i], DIMS[i + 1]
        Ws.append(nc.declare_dram_parameter(f"W{i+1}", [fi, fo], DT, isOutput=False))
        bs.append(nc.declare_dram_parameter(f"b{i+1}", [fo, 1], F32, isOutput=False))
    b4r_d = nc.declare_dram_parameter("b4r", [1, 128], DT, isOutput=False)
    b5r_d = nc.declare_dram_parameter("b5r", [1, 2048], DT, isOutput=False)
    idx_lo_d = nc.declare_dram_parameter("idx_lo", [128, idx_lo_cols], I16, isOutput=False)
    idx_hi_d = nc.declare_dram_parameter("idx_hi", [128, idx_hi_cols], I16, isOutput=False)
    s_lo_d = nc.declare_dram_parameter("s_lo", [P, tot_clo, P], DT, isOutput=False)
    s_hi_d = nc.declare_dram_parameter("s_hi", [P, tot_chi, P], DT, isOutput=False)
    out_d = nc.declare_dram_parameter("out", [NPC, 2048], DT, isOutput=True)

    with tile.TileContext(nc) as tc:
        with (
            tc.tile_pool(name="dram", bufs=1, space="DRAM") as dram,
            tc.tile_pool(name="cpool", bufs=1) as cpool,
            tc.tile_pool(name="sb", bufs=2) as sb,
            tc.tile_pool(name="pagg", bufs=1, space="PSUM") as pagg,
            tc.tile_pool(name="pmm", bufs=4, space="PSUM") as pmm,
        ):
            # ---- internal DRAM ----
            g2_d = dram.tile([NPC, 512], DT)
            g3_d = dram.tile([NPC, 256], DT)
            g4_d = dram.tile([NPC, 128], DT)
            h4_d = dram.tile([NPC, 128], DT)
            # split tables: A = nodes [0, 25000), B = nodes [25000, 50000)
            T2a = dram.tile([HALF, 512], DT, addr_space="Shared")
            T2b = dram.tile([HALF, 512], DT, addr_space="Shared")
            T3a = dram.tile([HALF, 256], DT, addr_space="Shared")
            T3b = dram.tile([HALF, 256], DT, addr_space="Shared")
            T4a = dram.tile([HALF, 128], DT, addr_space="Shared")
            T4b = dram.tile([HALF, 128], DT, addr_space="Shared")
            T5a = dram.tile([HALF, 128], DT, addr_space="Shared")
            T5b = dram.tile([HALF, 128], DT, addr_space="Shared")

            # ---- resident SBUF ----
            ones_sb = cpool.tile([1, 128], DT, name="ones")
            nc.any.memset(ones_sb[:], 1.0)
            b4r_sb = cpool.tile([1, 128], DT, name="b4rsb")
            nc.sync.dma_start(b4r_sb[:], b4r_d[:])
            b5r_sb = cpool.tile([1, 2048], DT, name="b5rsb")
            nc.sync.dma_start(b5r_sb[:], b5r_d[:])
            idx_lo_sb = cpool.tile([128, idx_lo_cols], I16, name="idxlo")
            nc.sync.dma_start(idx_lo_sb[:], idx_lo_d[:])
            idx_hi_sb = cpool.tile([128, idx_hi_cols], I16, name="idxhi")
            nc.sync.dma_start(idx_hi_sb[:], idx_hi_d[:])

            rg = [list(range(NC))]

            def load_w(i):
                fi, fo = DIMS[i], DIMS[i + 1]
                ks = []
                for k in range(_ceil_div(fi, P)):
                    kk = min(P, fi - k * P)
                    t_ = cpool.tile([P, fo], DT, name=f"w{i}_{k}", tag=f"w{i}k{k}")
                    nc.sync.dma_start(t_[:kk, :], Ws[i][k * P : k * P + kk, :])
                    ks.append((t_, kk))
                return ks

            def load_bcol(i):
                fo = DIMS[i + 1]
                nchunk = _ceil_div(fo, P)
                t_ = cpool.tile([P, 16], F32, name=f"bc{i}", tag=f"bcol{i}")
                for m in range(nchunk):
                    mm = min(P, fo - m * P)
                    nc.sync.dma_start(t_[:mm, m : m + 1], bs[i][m * P : m * P + mm, :])
                return t_

            def allgather2(src_d, dst_a, dst_b):
                nc.gpsimd.collective_compute(
                    "AllGather", mybir.AluOpType.bypass, replica_groups=rg,
                    ins=[src_d[:HPC, :].opt()], outs=[dst_a[:].opt()],
                )
                nc.gpsimd.collective_compute(
                    "AllGather", mybir.AluOpType.bypass, replica_groups=rg,
                    ins=[src_d[HPC:, :].opt()], outs=[dst_b[:].opt()],
                )

            # ================= aggregation =================
            def aggregate(layer, tab_a, tab_b, out_cb, node_major=False):
                """Gather + aggregate all dst tiles.

                layer 0 reads host-shipped pre-gathered x rows via plain DMA;
                other layers dma_gather rows from tab_a (src<25000) / tab_b.
                16-granular schedules, partial-K matmuls on last chunks.
                out_cb(t, tw, pts): per-F-chunk PSUM banks for dst tile t.
                """
                fa = LAYER_FA[layer]
                fap = LAYER_FA_PAD[layer]
                nfc = _ceil_div(fa, P)
                LAG = 0 if layer == 0 else 2

                def emit_front(g0):
                    tiles = list(range(g0, min(g0 + 4, N_TILES)))
                    t0, t1 = tiles[0], tiles[-1]
                    clo_g = int(offc_lo[t1 + 1] - offc_lo[t0])
                    g_chi = int(offc_hi[t1 + 1] - offc_hi[t0])
                    slo_sb = sb.tile([128, clo_g, P], DT, bufs=3,
                                     name=f"slo_{layer}_{g0}", tag="slo")
                    nc.sync.dma_start(
                        slo_sb[:], s_lo_d[:, offc_lo[t0] : offc_lo[t1 + 1], :])
                    shi_sb = None
                    if g_chi > 0:
                        shi_sb = sb.tile([128, g_chi, P], DT, name=f"shi_{layer}_{g0}", tag="shi")
                        nc.sync.dma_start(
                            shi_sb[:], s_hi_d[:, offc_hi[t0] : offc_hi[t1 + 1], :])
                    if layer == 0:
                        hg_lo_g = sb.tile([128, clo_g, fap], DT, bufs=4, name=f"hglo_{layer}_{g0}", tag="hglo")
                        nc.sync.dma_start(
                            hg_lo_g[:], xg_lo_d[:, offc_lo[t0] : offc_lo[t1 + 1], :])
                        hg_hi_g = None
                        if g_chi > 0:
                            hg_hi_g = sb.tile([128, g_chi, fap], DT, name=f"hghi_{layer}_{g0}", tag="hghi")
                            nc.sync.dma_start(
                                hg_hi_g[:], xg_hi_d[:, offc_hi[t0] : offc_hi[t1 + 1], :])
                    else:
                        # batched lo gathers (Q7 fixed cost is per call);
                        # scheds are 128-granular so tile chunks align
                        hg_lo_g = sb.tile([128, clo_g, fap], DT, bufs=4,
                                          name=f"hglo_{layer}_{g0}", tag="hglo")
                        for s0 in range(0, len(tiles), GS):
                            ta, tb = tiles[s0], min(tiles[-1], tiles[s0] + GS - 1)
                            ni = int(sched_lo[ta : tb + 1].sum())
                            c_a = int(offc_lo[ta] - offc_lo[t0])
                            c_b = int(offc_lo[tb + 1] - offc_lo[t0])
                            nc.gpsimd.dma_gather(
                                hg_lo_g[:, c_a:c_b, :], tab_a,
                                idx_lo_sb[:, offi_lo[ta] : offi_lo[tb + 1]],
                                ni, ni, fap,
                            )
                        hg_hi_g = "defer"
                    return (tiles, t0, t1, g_chi, slo_sb, shi_sb, hg_lo_g, hg_hi_g)

                def emit_back(st):
                    tiles, t0, t1, g_chi, slo_sb, shi_sb, hg_lo_g, hg_hi_g = st
                    if hg_hi_g == "defer":
                        # deferred hi gathers (wait on the late AG half)
                        hg_hi_g = None
                        if int(sched_hi[t0 : t1 + 1].sum()) > 0:
                            hg_hi_g = sb.tile([128, g_chi, fap], DT,
                                              name=f"hghi_{layer}_{t0}", tag="hghi")
                            for s0 in range(0, len(tiles), GS):
                                ta, tb = tiles[s0], min(tiles[-1], tiles[s0] + GS - 1)
                                ni = int(sched_hi[ta : tb + 1].sum())
                                if ni == 0:
                                    continue
                                c_a = int(offc_hi[ta] - offc_hi[t0])
                                c_b = int(offc_hi[tb + 1] - offc_hi[t0])
                                nc.gpsimd.dma_gather(
                                    hg_hi_g[:, c_a:c_b, :], tab_b,
                                    idx_hi_sb[:, offi_hi[ta] : offi_hi[tb + 1]],
                                    ni, ni, fap,
                                )
                    do_tiles(tiles, t0, slo_sb, shi_sb, hg_lo_g, hg_hi_g)

                def do_tiles(tiles, t0, slo_sb, shi_sb, hg_lo_g, hg_hi_g):
                    for t in tiles:
                        tw = TILE_WIDTHS[t]
                        chunks = []
                        for ci in range(int(ch_lo[t])):
                            c = int(offc_lo[t] - offc_lo[t0]) + ci
                            chunks.append((hg_lo_g, c, slo_sb, c, P))
                        for ci in range(int(ch_hi[t])):
                            c = int(offc_hi[t] - offc_hi[t0]) + ci
                            chunks.append((hg_hi_g, c, shi_sb, c, P))
                        # one PSUM bank per accumulation group (first_mm's
                        # has_written clear is partition-row x bank granular)
                        pts = [
                            pagg.tile([P, P], F32, name=f"pt_{layer}_{t}_{fc}",
                                      tag=f"pagg{fc}", space="PSUM", bufs=1)
                            for fc in range(nfc)
                        ]
                        nch = len(chunks)
                        if node_major:
                            for ci, (hg, hc, ssb, sc, K) in enumerate(chunks):
                                nc.tensor.matmul(
                                    pts[0][:, :fa], ssb[:K, sc, :], hg[:K, hc, :fa],
                                    start=(ci == 0), stop=False,
                                )
                            nc.tensor.matmul(  # += bias row
                                pts[0][:, :fa], ones_sb[:1, :], b4r_sb[:1, :fa],
                                start=False, stop=True,
                            )
                        else:
                            for ci, (hg, hc, ssb, sc, K) in enumerate(chunks):
                                for fc in range(nfc):
                                    fw = min(P, fa - fc * P)
                                    nc.tensor.matmul(
                                        pts[fc][:fw, :],
                                        hg[:K, hc, fc * P : fc * P + fw],
                                        ssb[:K, sc, :],
                                        start=(ci == 0), stop=(ci == nch - 1),
                                    )
                        out_cb(t, tw, pts)

                pending = []
                for g0 in range(0, N_TILES, 4):
                    pending.append(emit_front(g0))
                    if len(pending) > LAG:
                        emit_back(pending.pop(0))
                while pending:
                    emit_back(pending.pop(0))

            # ============ fused window pipeline ============
            # Per 4-tile group (<=512 contiguous local nodes): agg psums are
            # evicted into F-major stage tiles (SBUF), the NEXT layer's dense
            # consumes them directly (activations never round-trip DRAM), the
            # dense output g window is written to DRAM, and the AllGather
            # halves fire as soon as their input rows are complete.
            w1 = load_w(0)
            b1c = load_bcol(0)
            w2 = load_w(1)
            b2c = load_bcol(1)
            w3 = load_w(2)
            b3c = load_bcol(2)
            w4 = load_w(3)
            w5 = load_w(4)

            def make_ag(src_d, dst_a, dst_b):
                def ag(which):
                    if which == 0:
                        nc.gpsimd.collective_compute(
                            "AllGather", mybir.AluOpType.bypass, replica_groups=rg,
                            ins=[src_d[:HPC, :].opt()], outs=[dst_a[:].opt()],
                        )
                    else:
                        nc.gpsimd.collective_compute(
                            "AllGather", mybir.AluOpType.bypass, replica_groups=rg,
                            ins=[src_d[HPC:, :].opt()], outs=[dst_b[:].opt()],
                        )
                return ag

            def make_dense_window(li, w_tiles, nk, fo, g_dst):
                # node-major dense from F-major stage chunks (lhsT = stages)
                def dense_fn(c0, cols, stages):
                    for m0 in range(0, cols, P):
                        mw = min(P, cols - m0)
                        pm = pmm.tile([P, 512], F32, name=f"pm_{li}_{c0}_{m0}",
                                      tag="pmm", space="PSUM")
                        for k in range(nk):
                            kk = w_tiles[k][1]
                            nc.tensor.matmul(
                                pm[:mw, :fo],
                                stages[k][:kk, m0 : m0 + mw],
                                w_tiles[k][0][:kk, :fo],
                                start=(k == 0), stop=(k == nk - 1),
                            )
                        ev = sb.tile([P, 512], DT, name=f"ev_{li}_{c0}_{m0}", tag="ev")
                        nc.vector.tensor_copy(ev[:mw, :fo], pm[:mw, :fo])
                        nc.sync.dma_start(
                            g_dst[c0 + m0 : c0 + m0 + mw, :fo], ev[:mw, :fo])
                return dense_fn

            def l1_window(c0, cols, stages0):
                # L1 dense (F-major h1 chunks, lrelu+b1) then L2 dense -> g2
                h1st = [sb.tile([P, 512], DT, name=f"h1_{c0}_{m}", tag=f"h1st{m}")
                        for m in range(8)]
                for m in range(8):
                    pm = pmm.tile([P, 512], F32, name=f"apm_{c0}_{m}",
                                  tag="pmm", space="PSUM")
                    for k in range(3):
                        kk = (128, 128, 44)[k]
                        nc.tensor.matmul(
                            pm[:, :cols],
                            w1[k][0][:kk, m * P : (m + 1) * P],
                            stages0[k][:kk, :cols],
                            start=(k == 0), stop=(k == 2),
                        )
                    nc.scalar.activation(
                        h1st[m][:, :cols], pm[:, :cols], LRELU,
                        bias=b1c[:, m : m + 1], alpha=NEG_SLOPE,
                    )
                make_dense_window(2, w2, 8, 512, g2_d)(c0, cols, h1st)

            def out_window(c0, cols, stages):
                for m0 in range(0, cols, P):
                    mw = min(P, cols - m0)
                    ev = sb.tile([P, 2048], DT, name=f"oev_{c0}_{m0}", tag="oev")
                    for n in range(4):
                        pm = pmm.tile([P, 512], F32, name=f"pm5_{c0}_{m0}_{n}",
                                      tag="pmm", space="PSUM")
                        nc.tensor.matmul(
                            pm[:mw, :], stages[0][:, m0 : m0 + mw],
                            w5[0][0][:, n * 512 : (n + 1) * 512],
                            start=True, stop=False,
                        )
                        nc.tensor.matmul(  # += bias row (K=1 outer product)
                            pm[:mw, :], ones_sb[:1, :mw],
                            b5r_sb[:1, n * 512 : (n + 1) * 512],
                            start=False, stop=True,
                        )
                        nc.vector.tensor_copy(
                            ev[:mw, n * 512 : (n + 1) * 512], pm[:mw, :])
                    nc.sync.dma_start(out_d[c0 + m0 : c0 + m0 + mw, :], ev[:mw, :])

            def make_fused_out(layer, nfc, fa, bias_col, lrelu, window_fn, ag_fn):
                state = {"stages": None, "c0": 0, "col": 0, "ag_a": False}

                def cb(t, tw, pts):
                    if t % 4 == 0:
                        state["stages"] = [
                            sb.tile([P, 512], DT, name=f"st_{layer}_{t}_{fc}", tag=f"st{fc}")
                            for fc in range(nfc)
                        ]
                        state["c0"] = TILE_STARTS[t]
                        state["col"] = 0
                    col = state["col"]
                    for fc in range(nfc):
                        fw = min(P, fa - fc * P)
                        if lrelu:
                            nc.scalar.activation(
                                state["stages"][fc][:fw, col : col + tw],
                                pts[fc][:fw, :tw],
                                LRELU, bias=bias_col[:, fc : fc + 1], alpha=NEG_SLOPE,
                            )
                        else:
                            nc.vector.tensor_copy(
                                state["stages"][fc][:fw, col : col + tw],
                                pts[fc][:fw, :tw],
                            )
                    state["col"] = col + tw
                    if t % 4 == 3 or t == N_TILES - 1:
                        window_fn(state["c0"], state["col"], state["stages"])
                        if ag_fn is not None:
                            covered = state["c0"] + state["col"]
                            if not state["ag_a"] and covered >= HPC:
                                ag_fn(0)
                                state["ag_a"] = True
                            if t == N_TILES - 1:
                                ag_fn(1)

                return cb

            # ================= the network =================
            # L1: aggregate x rows -> [L1 dense -> L2 dense] per window -> g2
            aggregate(0, None, None,
                      make_fused_out(0, 3, 300, None, False, l1_window,
                                     make_ag(g2_d, T2a, T2b)))

            # L2: aggregate g2 (Lrelu+b2) -> L3 dense -> g3, AG3
            aggregate(1, T2a[:, :], T2b[:, :],
                      make_fused_out(1, 4, 512, b2c, True,
                                     make_dense_window(3, w3, 4, 256, g3_d),
                                     make_ag(g3_d, T3a, T3b)))

            # L3: aggregate g3 (Lrelu+b3) -> L4 dense -> g4, AG4
            aggregate(2, T3a[:, :], T3b[:, :],
                      make_fused_out(2, 2, 256, b3c, True,
                                     make_dense_window(4, w4, 2, 128, g4_d),
                                     make_ag(g4_d, T4a, T4b)))

            # L4: aggregate g4 node-major (+b4 via matmul, Lrelu) -> h4, AG5
            ag5 = make_ag(h4_d, T5a, T5b)
            l4_state = {"ag_a": False}

            def l4_out(t, tw, pts):
                ev = sb.tile([P, 512], DT, name=f"l4ev_{t}", tag="ev")
                nc.scalar.activation(ev[:tw, :128], pts[0][:tw, :128], LRELU, alpha=NEG_SLOPE)
                nc.scalar.dma_start(
                    h4_d[TILE_STARTS[t] : TILE_STARTS[t] + tw, :], ev[:tw, :128])
                covered = TILE_STARTS[t] + tw
                if not l4_state["ag_a"] and covered >= HPC:
                    ag5(0)
                    l4_state["ag_a"] = True
                if t == N_TILES - 1:
                    ag5(1)

            aggregate(3, T4a[:, :], T4b[:, :], l4_out, node_major=True)

            # L5: aggregate h4 -> out dense (W5 + b5) per window -> out
            aggregate(4, T5a[:, :], T5b[:, :],
                      make_fused_out(4, 1, 128, None, False, out_window, None))

    nc.compile()
    return nc


# ----------------------------------------------------------------------------
# Entry point
# ----------------------------------------------------------------------------

_CACHE = {}


def _run(inputs, trace=False):
    x = np.asarray(inputs["x"], dtype=np.float32)
    edge_index = np.asarray(inputs["edge_index"])
    sched_lo, sched_hi, per_core = _prep(edge_index, x)

    key = (tuple(sched_lo.tolist()), tuple(sched_hi.tolist()))
    if key not in _CACHE:
        _CACHE[key] = _build(sched_lo, sched_hi)
    nc = _CACHE[key]

    common = {}
    for i in range(5):
        common[f"W{i+1}"] = np.ascontiguousarray(
            np.asarray(inputs[f"W{i+1}"], dtype=np.float32).astype(NPDT))
        common[f"b{i+1}"] = np.ascontiguousarray(
            np.asarray(inputs[f"b{i+1}"], dtype=np.float32).reshape(-1, 1))
    common["b4r"] = np.ascontiguousarray(common["b4"].reshape(1, 128).astype(NPDT))
    common["b5r"] = np.ascontiguousarray(
        np.asarray(inputs["b5"], dtype=np.float32).reshape(1, 2048).astype(NPDT))

    in_maps = [
        {**common, **{k: (v.astype(NPDT) if k.startswith("s_") else v)
                      for k, v in per_core[c].items()}}
        for c in range(NC)
    ]
    res = run_bass_kernel_spmd(nc, in_maps, core_ids=list(range(NC)), trace=trace)
    # reassemble: core c rows [0:HPC] -> global [c*HPC:(c+1)*HPC],
    #             rows [HPC:NPC] -> global [HALF + c*HPC : HALF + (c+1)*HPC]
    out = np.empty((N_NODES, 2048), dtype=np.float32)
    for c in range(NC):
        oc = np.asarray(res.results[c]["out"], dtype=np.float32)
        out[c * HPC : (c + 1) * HPC] = oc[:HPC]
        out[HALF + c * HPC : HALF + (c + 1) * HPC] = oc[HPC:]
    return out, res


def kernel(**inputs):
    out, _ = _run(inputs, trace=False)
    return out



# revision 31
# speedup vs baseline: 1.0338x; 1.0338x over previous
"""5-layer DGL-style GraphConv (AwA2Conv) on 8 Trainium2 NeuronCores.

Math per layer (norm='both'):
    out = D_in^{-1/2} A D_out^{-1/2} (h) @ W + b     (+ leaky_relu except last)

The per-edge weight w_e = dinv_out[src]*dinv_in[dst] is folded into
block-sparse "S" matrices (128-edge x 128-dst chunks) so the sparse
aggregation becomes PE matmuls over dma_gather'ed edge rows. Aggregation
runs at min(Fin, Fout) per layer (matmul commutes with the linear
aggregation). With lhsT = gathered rows the aggregate comes out TRANSPOSED
[F, dst] — exactly the lhsT layout the next dense matmul wants, so the
network runs with zero explicit transposes.

Distribution: dual-block node sharding — core c owns global nodes
[c*3125,(c+1)*3125) u [25000+c*3125, 25000+(c+1)*3125). Each activation
exchange is TWO AllGathers (node halves A/B); gathers for edges with
src<25000 read table A and only wait on the first collective, overlapping
the second. Layer-1 edge rows (gathered from the replicated input x) are
materialized host-side and shipped per core.
"""

import numpy as np
import ml_dtypes

import concourse.bass as bass
import concourse.bacc as bacc
import concourse.mybir as mybir
import concourse.tile as tile
from concourse.bass_utils import run_bass_kernel_spmd

N_NODES = 50000
N_EDGES = 250000
NC = 8
NPC = N_NODES // NC      # 6250 nodes per core
HALF = 25000             # global half boundary (= lo/hi gather split)
HPC = HALF // NC         # 3125 nodes per core per half
P = 128
TPH = 25                 # tiles per half (24x128 + 1x53)
N_TILES = 2 * TPH        # 50 dst tiles per core
DIMS = [300, 1024, 512, 256, 128, 2048]
NEG_SLOPE = 0.01

F32 = mybir.dt.float32
BF16 = mybir.dt.bfloat16
DT = BF16
NPDT = ml_dtypes.bfloat16
I16 = mybir.dt.int16
LRELU = mybir.ActivationFunctionType.Lrelu

LAYER_FA = [300, 512, 256, 128, 128]       # aggregation width
LAYER_FA_PAD = [384, 512, 256, 128, 128]   # gathered row width (256B mult)
GS = 2                                     # tiles per dma_gather call


def _ceil_div(a, b):
    return (a + b - 1) // b


def _tile_start(t):
    return (t // TPH) * HPC + (t % TPH) * P


def _tile_width(t):
    return HPC - (TPH - 1) * P if (t % TPH) == TPH - 1 else P


TILE_STARTS = [_tile_start(t) for t in range(N_TILES)]
TILE_WIDTHS = [_tile_width(t) for t in range(N_TILES)]


# ----------------------------------------------------------------------------
# Host-side graph preprocessing
# ----------------------------------------------------------------------------

def _prep(edge_index, x):
    """Partition edges by (dst core, dst tile), split by src half, pad to
    16-granular per-tile schedules (max across cores -> one SPMD program).

    Returns (sched_lo, sched_hi, per_core). per_core: wrapped int16 gather
    indices, S chunk matrices, and pre-gathered layer-1 x rows.
    """
    GRAN = 128
    src = np.asarray(edge_index[0], dtype=np.int64)
    dst = np.asarray(edge_index[1], dtype=np.int64)
    out_deg = np.bincount(src, minlength=N_NODES).astype(np.float32)
    in_deg = np.bincount(dst, minlength=N_NODES).astype(np.float32)
    dinv_out = 1.0 / np.sqrt(np.maximum(out_deg, 1.0))
    dinv_in = 1.0 / np.sqrt(np.maximum(in_deg, 1.0))
    w = (dinv_out[src] * dinv_in[dst]).astype(np.float32)
    xb = np.asarray(x, dtype=np.float32)

    # dst -> (core, local pos); dual-block sharding
    d_half = dst // HALF
    d_rem = dst % HALF
    d_core = d_rem // HPC
    d_with = d_rem % HPC
    d_pos = d_with + d_half * HPC               # local position in [0, NPC)
    d_tile = d_half * TPH + np.minimum(d_with // P, TPH - 1)
    lo = src < HALF

    key = (d_core * N_TILES + d_tile) * 2 + (~lo).astype(np.int64)
    order = np.lexsort((src, key))
    src_s, w_s, pos_s, key_s = src[order], w[order], d_pos[order], key[order]
    bounds = np.searchsorted(key_s, np.arange(NC * N_TILES * 2 + 1))

    n_lo = np.zeros((NC, N_TILES), dtype=np.int64)
    n_hi = np.zeros((NC, N_TILES), dtype=np.int64)
    for c in range(NC):
        for t in range(N_TILES):
            k = (c * N_TILES + t) * 2
            n_lo[c, t] = bounds[k + 1] - bounds[k]
            n_hi[c, t] = bounds[k + 2] - bounds[k + 1]

    sched_lo = np.maximum(
        np.ceil(n_lo.max(axis=0) / GRAN).astype(np.int64), 1) * GRAN
    sched_hi = np.ceil(n_hi.max(axis=0) / GRAN).astype(np.int64) * GRAN

    per_core = []
    for c in range(NC):
        idx_parts = {True: [], False: []}
        s_parts = {True: [], False: []}
        # host-side L1 aggregation (the x rows are host-staged anyway):
        # agg1[d] = sum_e w_e x[src_e], shipped F-major padded to 384
        k0, k1 = bounds[c * N_TILES * 2], bounds[(c + 1) * N_TILES * 2]
        p_all = pos_s[k0:k1]
        rows = (w_s[k0:k1, None] * xb[src_s[k0:k1]]).astype(np.float32)
        o = np.argsort(p_all, kind="stable")
        p_sorted = p_all[o]
        uniq = np.unique(p_sorted)
        seg = np.searchsorted(p_sorted, uniq)
        sums = np.add.reduceat(rows[o], seg, axis=0)
        agg1 = np.zeros((NPC, 300), dtype=np.float32)
        agg1[uniq] = sums
        agg1T = np.zeros((384, NPC), dtype=NPDT)
        agg1T[:300] = agg1.T.astype(NPDT)
        for t in range(N_TILES):
            k = (c * N_TILES + t) * 2
            segs = (
                (True, sched_lo[t], bounds[k], bounds[k + 1]),
                (False, sched_hi[t], bounds[k + 1], bounds[k + 2]),
            )
            for islo, ni, a, b_ in segs:
                ni = int(ni)
                if ni == 0:
                    continue
                n_slots = _ceil_div(ni, P) * P
                ne = b_ - a
                assert ne <= ni
                idx = np.zeros(ni, dtype=np.int64)
                idx[:ne] = src_s[a:b_] - (0 if islo else HALF)
                dstloc = np.full(n_slots, P, dtype=np.int64)
                dstloc[:ne] = pos_s[a:b_] - TILE_STARTS[t]
                wv = np.zeros(n_slots, dtype=np.float32)
                wv[:ne] = w_s[a:b_]
                S = np.zeros((n_slots, P), dtype=np.float32)
                valid = dstloc < P
                S[np.nonzero(valid)[0], dstloc[valid]] = wv[valid]
                s_parts[islo].append(S.reshape(-1, P, P))
                idx_parts[islo].append(idx.reshape(-1, 16).T.astype(np.int16))
        pc = {"agg1T": np.ascontiguousarray(agg1T)}
        for islo, nm in ((True, "lo"), (False, "hi")):
            if idx_parts[islo]:
                pc[f"idx_{nm}"] = np.ascontiguousarray(
                    np.tile(np.concatenate(idx_parts[islo], axis=1), (8, 1)))
                # store S partition-major ([P, c, n]) so the group DMA
                # reads each partition contiguously (no strided rearrange)
                pc[f"s_{nm}"] = np.ascontiguousarray(
                    np.concatenate(s_parts[islo], axis=0).transpose(1, 0, 2))
            else:
                pc[f"idx_{nm}"] = np.zeros((128, 1), dtype=np.int16)
                pc[f"s_{nm}"] = np.zeros((P, 1, P), dtype=np.float32)
        per_core.append(pc)
    return sched_lo, sched_hi, per_core


# ----------------------------------------------------------------------------
# Bass program builder (depends only on sched_lo / sched_hi)
# ----------------------------------------------------------------------------

def _build(sched_lo, sched_hi):
    nc = bacc.Bacc("TRN2")
    ch_lo = np.ceil(sched_lo / P).astype(np.int64)
    ch_hi = np.ceil(sched_hi / P).astype(np.int64)
    idx_lo_cols = int(sched_lo.sum()) // 16
    idx_hi_cols = max(int(sched_hi.sum()) // 16, 1)
    tot_clo = int(ch_lo.sum())
    tot_chi = max(int(ch_hi.sum()), 1)
    offi_lo = np.concatenate([[0], np.cumsum(sched_lo // 16)]).astype(int)
    offi_hi = np.concatenate([[0], np.cumsum(sched_hi // 16)]).astype(int)
    offc_lo = np.concatenate([[0], np.cumsum(ch_lo)]).astype(int)
    offc_hi = np.concatenate([[0], np.cumsum(ch_hi)]).astype(int)

    agg1T_d = nc.declare_dram_parameter("agg1T", [384, NPC], DT, isOutput=False)
    Ws, bs = [], []
    for i in range(5):
        fi, fo = DIMS[i], DIMS[i + 1]
        Ws.append(nc.declare_dram_parameter(f"W{i+1}", [fi, fo], DT, isOutput=False))
        bs.append(nc.declare_dram_parameter(f"b{i+1}", [fo, 1], F32, isOutput=False))
    b4r_d = nc.declare_dram_parameter("b4r", [1, 128], DT, isOutput=False)
    b5r_d = nc.declare_dram_parameter("b5r", [1, 2048], DT, isOutput=False)
    idx_lo_d = nc.declare_dram_parameter("idx_lo", [128, idx_lo_cols], I16, isOutput=False)
    idx_hi_d = nc.declare_dram_parameter("idx_hi", [128, idx_hi_cols], I16, isOutput=False)
    s_lo_d = nc.declare_dram_parameter("s_lo", [P, tot_clo, P], DT, isOutput=False)
    s_hi_d = nc.declare_dram_parameter("s_hi", [P, tot_chi, P], DT, isOutput=False)
    out_d = nc.declare_dram_parameter("out", [NPC, 2048], DT, isOutput=True)

    with tile.TileContext(nc) as tc:
        with (
            tc.tile_pool(name="dram", bufs=1, space="DRAM") as dram,
            tc.tile_pool(name="cpool", bufs=1) as cpool,
            tc.tile_pool(name="sb", bufs=2) as sb,
            tc.tile_pool(name="pagg", bufs=1, space="PSUM") as pagg,
            tc.tile_pool(name="pmm", bufs=4, space="PSUM") as pmm,
        ):
            # ---- internal DRAM ----
            g2_d = dram.tile([NPC, 512], DT)
            g3_d = dram.tile([NPC, 256], DT)
            g4_d = dram.tile([NPC, 128], DT)
            h4_d = dram.tile([NPC, 128], DT)
            # split tables: A = nodes [0, 25000), B = nodes [25000, 50000)
            T2a = dram.tile([HALF, 512], DT, addr_space="Shared")
            T2b = dram.tile([HALF, 512], DT, addr_space="Shared")
            T3a = dram.tile([HALF, 256], DT, addr_space="Shared")
            T3b = dram.tile([HALF, 256], DT, addr_space="Shared")
            T4a = dram.tile([HALF, 128], DT, addr_space="Shared")
            T4b = dram.tile([HALF, 128], DT, addr_space="Shared")
            T5a = dram.tile([HALF, 128], DT, addr_space="Shared")
            T5b = dram.tile([HALF, 128], DT, addr_space="Shared")

            # ---- resident SBUF ----
            ones_sb = cpool.tile([1, 128], DT, name="ones")
            nc.any.memset(ones_sb[:], 1.0)
            b4r_sb = cpool.tile([1, 128], DT, name="b4rsb")
            nc.sync.dma_start(b4r_sb[:], b4r_d[:])
            b5r_sb = cpool.tile([1, 2048], DT, name="b5rsb")
            nc.sync.dma_start(b5r_sb[:], b5r_d[:])
            idx_lo_sb = cpool.tile([128, idx_lo_cols], I16, name="idxlo")
            nc.sync.dma_start(idx_lo_sb[:], idx_lo_d[:])
            idx_hi_sb = cpool.tile([128, idx_hi_cols], I16, name="idxhi")
            nc.sync.dma_start(idx_hi_sb[:], idx_hi_d[:])

            rg = [list(range(NC))]

            def load_w(i):
                fi, fo = DIMS[i], DIMS[i + 1]
                ks = []
                for k in range(_ceil_div(fi, P)):
                    kk = min(P, fi - k * P)
                    t_ = cpool.tile([P, fo], DT, name=f"w{i}_{k}", tag=f"w{i}k{k}")
                    nc.sync.dma_start(t_[:kk, :], Ws[i][k * P : k * P + kk, :])
                    ks.append((t_, kk))
                return ks

            def load_bcol(i):
                fo = DIMS[i + 1]
                nchunk = _ceil_div(fo, P)
                t_ = cpool.tile([P, 16], F32, name=f"bc{i}", tag=f"bcol{i}")
                for m in range(nchunk):
                    mm = min(P, fo - m * P)
                    nc.sync.dma_start(t_[:mm, m : m + 1], bs[i][m * P : m * P + mm, :])
                return t_

            def allgather2(src_d, dst_a, dst_b):
                nc.gpsimd.collective_compute(
                    "AllGather", mybir.AluOpType.bypass, replica_groups=rg,
                    ins=[src_d[:HPC, :].opt()], outs=[dst_a[:].opt()],
                )
                nc.gpsimd.collective_compute(
                    "AllGather", mybir.AluOpType.bypass, replica_groups=rg,
                    ins=[src_d[HPC:, :].opt()], outs=[dst_b[:].opt()],
                )

            # ================= aggregation =================
            def aggregate(layer, tab_a, tab_b, out_cb, node_major=False):
                """Gather + aggregate all dst tiles.

                layer 0 reads host-shipped pre-gathered x rows via plain DMA;
                other layers dma_gather rows from tab_a (src<25000) / tab_b.
                16-granular schedules, partial-K matmuls on last chunks.
                out_cb(t, tw, pts): per-F-chunk PSUM banks for dst tile t.
                """
                fa = LAYER_FA[layer]
                fap = LAYER_FA_PAD[layer]
                nfc = _ceil_div(fa, P)
                LAG = 0 if layer == 0 else 2

                def emit_front(g0):
                    tiles = list(range(g0, min(g0 + 4, N_TILES)))
                    t0, t1 = tiles[0], tiles[-1]
                    clo_g = int(offc_lo[t1 + 1] - offc_lo[t0])
                    g_chi = int(offc_hi[t1 + 1] - offc_hi[t0])
                    slo_sb = sb.tile([128, clo_g, P], DT, bufs=3,
                                     name=f"slo_{layer}_{g0}", tag="slo")
                    nc.sync.dma_start(
                        slo_sb[:], s_lo_d[:, offc_lo[t0] : offc_lo[t1 + 1], :])
                    shi_sb = None
                    if g_chi > 0:
                        shi_sb = sb.tile([128, g_chi, P], DT, name=f"shi_{layer}_{g0}", tag="shi")
                        nc.sync.dma_start(
                            shi_sb[:], s_hi_d[:, offc_hi[t0] : offc_hi[t1 + 1], :])
                    if True:
                        # batched lo gathers (Q7 fixed cost is per call);
                        # scheds are 128-granular so tile chunks align
                        hg_lo_g = sb.tile([128, clo_g, fap], DT, bufs=4,
                                          name=f"hglo_{layer}_{g0}", tag="hglo")
                        for s0 in range(0, len(tiles), GS):
                            ta, tb = tiles[s0], min(tiles[-1], tiles[s0] + GS - 1)
                            ni = int(sched_lo[ta : tb + 1].sum())
                            c_a = int(offc_lo[ta] - offc_lo[t0])
                            c_b = int(offc_lo[tb + 1] - offc_lo[t0])
                            nc.gpsimd.dma_gather(
                                hg_lo_g[:, c_a:c_b, :], tab_a,
                                idx_lo_sb[:, offi_lo[ta] : offi_lo[tb + 1]],
                                ni, ni, fap,
                            )
                        hg_hi_g = "defer"
                    return (tiles, t0, t1, g_chi, slo_sb, shi_sb, hg_lo_g, hg_hi_g)

                def emit_back(st):
                    tiles, t0, t1, g_chi, slo_sb, shi_sb, hg_lo_g, hg_hi_g = st
                    if hg_hi_g == "defer":
                        # deferred hi gathers (wait on the late AG half)
                        hg_hi_g = None
                        if int(sched_hi[t0 : t1 + 1].sum()) > 0:
                            hg_hi_g = sb.tile([128, g_chi, fap], DT,
                                              name=f"hghi_{layer}_{t0}", tag="hghi")
                            for s0 in range(0, len(tiles), GS):
                                ta, tb = tiles[s0], min(tiles[-1], tiles[s0] + GS - 1)
                                ni = int(sched_hi[ta : tb + 1].sum())
                                if ni == 0:
                                    continue
                                c_a = int(offc_hi[ta] - offc_hi[t0])
                                c_b = int(offc_hi[tb + 1] - offc_hi[t0])
                                nc.gpsimd.dma_gather(
                                    hg_hi_g[:, c_a:c_b, :], tab_b,
                                    idx_hi_sb[:, offi_hi[ta] : offi_hi[tb + 1]],
                                    ni, ni, fap,
                                )
                    do_tiles(tiles, t0, slo_sb, shi_sb, hg_lo_g, hg_hi_g)

                def do_tiles(tiles, t0, slo_sb, shi_sb, hg_lo_g, hg_hi_g):
                    for t in tiles:
                        tw = TILE_WIDTHS[t]
                        chunks = []
                        for ci in range(int(ch_lo[t])):
                            c = int(offc_lo[t] - offc_lo[t0]) + ci
                            chunks.append((hg_lo_g, c, slo_sb, c, P))
                        for ci in range(int(ch_hi[t])):
                            c = int(offc_hi[t] - offc_hi[t0]) + ci
                            chunks.append((hg_hi_g, c, shi_sb, c, P))
                        # one PSUM bank per accumulation group (first_mm's
                        # has_written clear is partition-row x bank granular)
                        pts = [
                            pagg.tile([P, P], F32, name=f"pt_{layer}_{t}_{fc}",
                                      tag=f"pagg{fc}", space="PSUM", bufs=1)
                            for fc in range(nfc)
                        ]
                        nch = len(chunks)
                        if node_major:
                            for ci, (hg, hc, ssb, sc, K) in enumerate(chunks):
                                nc.tensor.matmul(
                                    pts[0][:, :fa], ssb[:K, sc, :], hg[:K, hc, :fa],
                                    start=(ci == 0), stop=False,
                                )
                            nc.tensor.matmul(  # += bias row
                                pts[0][:, :fa], ones_sb[:1, :], b4r_sb[:1, :fa],
                                start=False, stop=True,
                            )
                        else:
                            for ci, (hg, hc, ssb, sc, K) in enumerate(chunks):
                                for fc in range(nfc):
                                    fw = min(P, fa - fc * P)
                                    nc.tensor.matmul(
                                        pts[fc][:fw, :],
                                        hg[:K, hc, fc * P : fc * P + fw],
                                        ssb[:K, sc, :],
                                        start=(ci == 0), stop=(ci == nch - 1),
                                    )
                        out_cb(t, tw, pts)

                pending = []
                for g0 in range(0, N_TILES, 4):
                    pending.append(emit_front(g0))
                    if len(pending) > LAG:
                        emit_back(pending.pop(0))
                while pending:
                    emit_back(pending.pop(0))

            # ============ fused window pipeline ============
            # Per 4-tile group (<=512 contiguous local nodes): agg psums are
            # evicted into F-major stage tiles (SBUF), the NEXT layer's dense
            # consumes them directly (activations never round-trip DRAM), the
            # dense output g window is written to DRAM, and the AllGather
            # halves fire as soon as their input rows are complete.
            w1 = load_w(0)
            b1c = load_bcol(0)
            w2 = load_w(1)
            b2c = load_bcol(1)
            w3 = load_w(2)
            b3c = load_bcol(2)
            w4 = load_w(3)
            w5 = load_w(4)

            def make_ag(src_d, dst_a, dst_b):
                def ag(which):
                    if which == 0:
                        nc.gpsimd.collective_compute(
                            "AllGather", mybir.AluOpType.bypass, replica_groups=rg,
                            ins=[src_d[:HPC, :].opt()], outs=[dst_a[:].opt()],
                        )
                    else:
                        nc.gpsimd.collective_compute(
                            "AllGather", mybir.AluOpType.bypass, replica_groups=rg,
                            ins=[src_d[HPC:, :].opt()], outs=[dst_b[:].opt()],
                        )
                return ag

            def make_dense_window(li, w_tiles, nk, fo, g_dst):
                # node-major dense from F-major stage chunks (lhsT = stages)
                def dense_fn(c0, cols, stages):
                    for m0 in range(0, cols, P):
                        mw = min(P, cols - m0)
                        pm = pmm.tile([P, 512], F32, name=f"pm_{li}_{c0}_{m0}",
                                      tag="pmm", space="PSUM")
                        for k in range(nk):
                            kk = w_tiles[k][1]
                            nc.tensor.matmul(
                                pm[:mw, :fo],
                                stages[k][:kk, m0 : m0 + mw],
                                w_tiles[k][0][:kk, :fo],
                                start=(k == 0), stop=(k == nk - 1),
                            )
                        ev = sb.tile([P, 512], DT, name=f"ev_{li}_{c0}_{m0}", tag="ev")
                        nc.vector.tensor_copy(ev[:mw, :fo], pm[:mw, :fo])
                        nc.sync.dma_start(
                            g_dst[c0 + m0 : c0 + m0 + mw, :fo], ev[:mw, :fo])
                return dense_fn

            def l1_window(c0, cols, stages0):
                # L1 dense (F-major h1 chunks, lrelu+b1) then L2 dense -> g2
                h1st = [sb.tile([P, 512], DT, name=f"h1_{c0}_{m}", tag=f"h1st{m}")
                        for m in range(8)]
                for m in range(8):
                    pm = pmm.tile([P, 512], F32, name=f"apm_{c0}_{m}",
                                  tag="pmm", space="PSUM")
                    for k in range(3):
                        kk = (128, 128, 44)[k]
                        nc.tensor.matmul(
                            pm[:, :cols],
                            w1[k][0][:kk, m * P : (m + 1) * P],
                            stages0[k][:kk, :cols],
                            start=(k == 0), stop=(k == 2),
                        )
                    nc.scalar.activation(
                        h1st[m][:, :cols], pm[:, :cols], LRELU,
                        bias=b1c[:, m : m + 1], alpha=NEG_SLOPE,
                    )
                make_dense_window(2, w2, 8, 512, g2_d)(c0, cols, h1st)

            def out_window(c0, cols, stages):
                for m0 in range(0, cols, P):
                    mw = min(P, cols - m0)
                    ev = sb.tile([P, 2048], DT, name=f"oev_{c0}_{m0}", tag="oev")
                    for n in range(4):
                        pm = pmm.tile([P, 512], F32, name=f"pm5_{c0}_{m0}_{n}",
                                      tag="pmm", space="PSUM")
                        nc.tensor.matmul(
                            pm[:mw, :], stages[0][:, m0 : m0 + mw],
                            w5[0][0][:, n * 512 : (n + 1) * 512],
                            start=True, stop=False,
                        )
                        nc.tensor.matmul(  # += bias row (K=1 outer product)
                            pm[:mw, :], ones_sb[:1, :mw],
                            b5r_sb[:1, n * 512 : (n + 1) * 512],
                            start=False, stop=True,
                        )
                        nc.vector.tensor_copy(
                            ev[:mw, n * 512 : (n + 1) * 512], pm[:mw, :])
                    nc.sync.dma_start(out_d[c0 + m0 : c0 + m0 + mw, :], ev[:mw, :])

            def make_fused_out(layer, nfc, fa, bias_col, lrelu, window_fn, ag_fn):
                state = {"stages": None, "c0": 0, "col": 0, "ag_a": False}

                def cb(t, tw, pts):
                    if t % 4 == 0:
                        state["stages"] = [
                            sb.tile([P, 512], DT, name=f"st_{layer}_{t}_{fc}", tag=f"st{fc}")
                            for fc in range(nfc)
                        ]
                        state["c0"] = TILE_STARTS[t]
                        state["col"] = 0
                    col = state["col"]
                    for fc in range(nfc):
                        fw = min(P, fa - fc * P)
                        if lrelu:
                            nc.scalar.activation(
                                state["stages"][fc][:fw, col : col + tw],
                                pts[fc][:fw, :tw],
                                LRELU, bias=bias_col[:, fc : fc + 1], alpha=NEG_SLOPE,
                            )
                        else:
                            nc.vector.tensor_copy(
                                state["stages"][fc][:fw, col : col + tw],
                                pts[fc][:fw, :tw],
                            )
                    state["col"] = col + tw
                    if t % 4 == 3 or t == N_TILES - 1:
                        window_fn(state["c0"], state["col"], state["stages"])
                        if ag_fn is not None:
                            covered = state["c0"] + state["col"]
                            if not state["ag_a"] and covered >= HPC:
                                ag_fn(0)
                                state["ag_a"] = True
                            if t == N_TILES - 1:
                                ag_fn(1)

                return cb

            # ================= the network =================
            # L1: host-aggregated x (agg1T) -> [L1 dense -> L2 dense] -> g2
            ag2 = make_ag(g2_d, T2a, T2b)
            ag2a_done = False
            for g0 in range(0, N_TILES, 4):
                tiles0 = list(range(g0, min(g0 + 4, N_TILES)))
                c0 = TILE_STARTS[tiles0[0]]
                cols = sum(TILE_WIDTHS[t] for t in tiles0)
                stages0 = [
                    sb.tile([P, 512], DT, name=f"a1_{g0}_{fc}", tag=f"st{fc}")
                    for fc in range(3)
                ]
                for fc in range(3):
                    fw = (128, 128, 44)[fc]
                    nc.sync.dma_start(
                        stages0[fc][:fw, :cols],
                        agg1T_d[fc * P : fc * P + fw, c0 : c0 + cols],
                    )
                l1_window(c0, cols, stages0)
                if not ag2a_done and c0 + cols >= HPC:
                    ag2(0)
                    ag2a_done = True
                if tiles0[-1] == N_TILES - 1:
                    ag2(1)

            # L2: aggregate g2 (Lrelu+b2) -> L3 dense -> g3, AG3
            aggregate(1, T2a[:, :], T2b[:, :],
                      make_fused_out(1, 4, 512, b2c, True,
                                     make_dense_window(3, w3, 4, 256, g3_d),
                                     make_ag(g3_d, T3a, T3b)))

            # L3: aggregate g3 (Lrelu+b3) -> L4 dense -> g4, AG4
            aggregate(2, T3a[:, :], T3b[:, :],
                      make_fused_out(2, 2, 256, b3c, True,
                                     make_dense_window(4, w4, 2, 128, g4_d),
                                     make_ag(g4_d, T4a, T4b)))

            # L4: aggregate g4 node-major (+b4 via matmul, Lrelu) -> h4, AG5
            ag5 = make_ag(h4_d, T5a, T5b)
            l4_state = {"ag_a": False}

            def l4_out(t, tw, pts):
                ev = sb.tile([P, 512], DT, name=f"l4ev_{t}", tag="ev")
                nc.scalar.activation(ev[:tw, :128], pts[0][:tw, :128], LRELU, alpha=NEG_SLOPE)
                nc.scalar.dma_start(
                    h4_d[TILE_STARTS[t] : TILE_STARTS[t] + tw, :], ev[:tw, :128])
                covered = TILE_STARTS[t] + tw
                if not l4_state["ag_a"] and covered >= HPC:
                    ag5(0)
                    l4_state["ag_a"] = True
                if t == N_TILES - 1:
                    ag5(1)

            aggregate(3, T4a[:, :], T4b[:, :], l4_out, node_major=True)

            # L5: aggregate h4 -> out dense (W5 + b5) per window -> out
            aggregate(4, T5a[:, :], T5b[:, :],
                      make_fused_out(4, 1, 128, None, False, out_window, None))

    nc.compile()
    return nc


# ----------------------------------------------------------------------------
# Entry point
# ----------------------------------------------------------------------------

_CACHE = {}


def _run(inputs, trace=False):
    x = np.asarray(inputs["x"], dtype=np.float32)
    edge_index = np.asarray(inputs["edge_index"])
    sched_lo, sched_hi, per_core = _prep(edge_index, x)

    key = (tuple(sched_lo.tolist()), tuple(sched_hi.tolist()))
    if key not in _CACHE:
        _CACHE[key] = _build(sched_lo, sched_hi)
    nc = _CACHE[key]

    common = {}
    for i in range(5):
        common[f"W{i+1}"] = np.ascontiguousarray(
            np.asarray(inputs[f"W{i+1}"], dtype=np.float32).astype(NPDT))
        common[f"b{i+1}"] = np.ascontiguousarray(
            np.asarray(inputs[f"b{i+1}"], dtype=np.float32).reshape(-1, 1))
    common["b4r"] = np.ascontiguousarray(common["b4"].reshape(1, 128).astype(NPDT))
    common["b5r"] = np.ascontiguousarray(
        np.asarray(inputs["b5"], dtype=np.float32).reshape(1, 2048).astype(NPDT))

    in_maps = [
        {**common, **{k: (v.astype(NPDT) if k.startswith("s_") else v)
                      for k, v in per_core[c].items()}}
        for c in range(NC)
    ]
    res = run_bass_kernel_spmd(nc, in_maps, core_ids=list(range(NC)), trace=trace)
    # reassemble: core c rows [0:HPC] -> global [c*HPC:(c+1)*HPC],
    #             rows [HPC:NPC] -> global [HALF + c*HPC : HALF + (c+1)*HPC]
    out = np.empty((N_NODES, 2048), dtype=np.float32)
    for c in range(NC):
        oc = np.asarray(res.results[c]["out"], dtype=np.float32)
        out[c * HPC : (c + 1) * HPC] = oc[:HPC]
        out[HALF + c * HPC : HALF + (c + 1) * HPC] = oc[HPC:]
    return out, res


def kernel(**inputs):
    out, _ = _run(inputs, trace=False)
    return out



# revision 32
# speedup vs baseline: 1.1579x; 1.1201x over previous
"""5-layer DGL-style GraphConv (AwA2Conv) on 8 Trainium2 NeuronCores.

Math per layer (norm='both'):
    out = D_in^{-1/2} A D_out^{-1/2} (h) @ W + b     (+ leaky_relu except last)

The per-edge weight w_e = dinv_out[src]*dinv_in[dst] is folded into
block-sparse "S" matrices (128-edge x 128-dst chunks) so the sparse
aggregation becomes PE matmuls over dma_gather'ed edge rows. Aggregation
runs at min(Fin, Fout) per layer (matmul commutes with the linear
aggregation). With lhsT = gathered rows the aggregate comes out TRANSPOSED
[F, dst] — exactly the lhsT layout the next dense matmul wants, so the
network runs with zero explicit transposes.

Distribution: dual-block node sharding — core c owns global nodes
[c*3125,(c+1)*3125) u [25000+c*3125, 25000+(c+1)*3125). Each activation
exchange is TWO AllGathers (node halves A/B); gathers for edges with
src<25000 read table A and only wait on the first collective, overlapping
the second. Layer-1 edge rows (gathered from the replicated input x) are
materialized host-side and shipped per core.
"""

import numpy as np
import ml_dtypes

import concourse.bass as bass
import concourse.bacc as bacc
import concourse.mybir as mybir
import concourse.tile as tile
from concourse.bass_utils import run_bass_kernel_spmd

N_NODES = 50000
N_EDGES = 250000
NC = 8
NPC = N_NODES // NC      # 6250 nodes per core
HALF = 25000             # global half boundary (= lo/hi gather split)
HPC = HALF // NC         # 3125 nodes per core per half
P = 128
TPH = 25                 # tiles per half (24x128 + 1x53)
N_TILES = 2 * TPH        # 50 dst tiles per core
DIMS = [300, 1024, 512, 256, 128, 2048]
NEG_SLOPE = 0.01

F32 = mybir.dt.float32
BF16 = mybir.dt.bfloat16
DT = BF16
NPDT = ml_dtypes.bfloat16
I16 = mybir.dt.int16
LRELU = mybir.ActivationFunctionType.Lrelu

LAYER_FA = [300, 512, 256, 128, 128]       # aggregation width
LAYER_FA_PAD = [384, 512, 256, 128, 128]   # gathered row width (256B mult)
GS = 2                                     # tiles per dma_gather call



MAX_CALL_IDXS = 896                        # proven-safe gather size (<1024)


def _pack_calls(tiles, sched):
    """Greedily pack consecutive tiles into gather calls <= MAX_CALL_IDXS."""
    calls = []
    i = 0
    while i < len(tiles):
        j = i
        tot = int(sched[tiles[i]])
        while j + 1 < len(tiles) and tot + int(sched[tiles[j + 1]]) <= MAX_CALL_IDXS:
            j += 1
            tot += int(sched[tiles[j]])
        calls.append((tiles[i], tiles[j]))
        i = j + 1
    return calls

def _ceil_div(a, b):
    return (a + b - 1) // b


def _tile_start(t):
    return (t // TPH) * HPC + (t % TPH) * P


def _tile_width(t):
    return HPC - (TPH - 1) * P if (t % TPH) == TPH - 1 else P


TILE_STARTS = [_tile_start(t) for t in range(N_TILES)]
TILE_WIDTHS = [_tile_width(t) for t in range(N_TILES)]


# ----------------------------------------------------------------------------
# Host-side graph preprocessing
# ----------------------------------------------------------------------------

def _prep(edge_index, x):
    """Partition edges by (dst core, dst tile), split by src half, pad to
    16-granular per-tile schedules (max across cores -> one SPMD program).

    Returns (sched_lo, sched_hi, per_core). per_core: wrapped int16 gather
    indices, S chunk matrices, and pre-gathered layer-1 x rows.
    """
    GRAN = 128
    src = np.asarray(edge_index[0], dtype=np.int64)
    dst = np.asarray(edge_index[1], dtype=np.int64)
    out_deg = np.bincount(src, minlength=N_NODES).astype(np.float32)
    in_deg = np.bincount(dst, minlength=N_NODES).astype(np.float32)
    dinv_out = 1.0 / np.sqrt(np.maximum(out_deg, 1.0))
    dinv_in = 1.0 / np.sqrt(np.maximum(in_deg, 1.0))
    w = (dinv_out[src] * dinv_in[dst]).astype(np.float32)
    xb = np.asarray(x, dtype=np.float32)

    # dst -> (core, local pos); dual-block sharding
    d_half = dst // HALF
    d_rem = dst % HALF
    d_core = d_rem // HPC
    d_with = d_rem % HPC
    d_pos = d_with + d_half * HPC               # local position in [0, NPC)
    d_tile = d_half * TPH + np.minimum(d_with // P, TPH - 1)
    lo = src < HALF

    key = (d_core * N_TILES + d_tile) * 2 + (~lo).astype(np.int64)
    order = np.lexsort((src, key))
    src_s, w_s, pos_s, key_s = src[order], w[order], d_pos[order], key[order]
    bounds = np.searchsorted(key_s, np.arange(NC * N_TILES * 2 + 1))

    n_lo = np.zeros((NC, N_TILES), dtype=np.int64)
    n_hi = np.zeros((NC, N_TILES), dtype=np.int64)
    for c in range(NC):
        for t in range(N_TILES):
            k = (c * N_TILES + t) * 2
            n_lo[c, t] = bounds[k + 1] - bounds[k]
            n_hi[c, t] = bounds[k + 2] - bounds[k + 1]

    sched_lo = np.maximum(
        np.ceil(n_lo.max(axis=0) / GRAN).astype(np.int64), 1) * GRAN
    sched_hi = np.ceil(n_hi.max(axis=0) / GRAN).astype(np.int64) * GRAN

    per_core = []
    for c in range(NC):
        idx_parts = {True: [], False: []}
        s_parts = {True: [], False: []}
        # host-side L1 aggregation (the x rows are host-staged anyway):
        # agg1[d] = sum_e w_e x[src_e], shipped F-major padded to 384
        k0, k1 = bounds[c * N_TILES * 2], bounds[(c + 1) * N_TILES * 2]
        p_all = pos_s[k0:k1]
        rows = (w_s[k0:k1, None] * xb[src_s[k0:k1]]).astype(np.float32)
        o = np.argsort(p_all, kind="stable")
        p_sorted = p_all[o]
        uniq = np.unique(p_sorted)
        seg = np.searchsorted(p_sorted, uniq)
        sums = np.add.reduceat(rows[o], seg, axis=0)
        agg1 = np.zeros((NPC, 300), dtype=np.float32)
        agg1[uniq] = sums
        agg1T = np.zeros((384, NPC), dtype=NPDT)
        agg1T[:300] = agg1.T.astype(NPDT)
        for t in range(N_TILES):
            k = (c * N_TILES + t) * 2
            segs = (
                (True, sched_lo[t], bounds[k], bounds[k + 1]),
                (False, sched_hi[t], bounds[k + 1], bounds[k + 2]),
            )
            for islo, ni, a, b_ in segs:
                ni = int(ni)
                if ni == 0:
                    continue
                n_slots = _ceil_div(ni, P) * P
                ne = b_ - a
                assert ne <= ni
                idx = np.zeros(ni, dtype=np.int64)
                idx[:ne] = src_s[a:b_] - (0 if islo else HALF)
                dstloc = np.full(n_slots, P, dtype=np.int64)
                dstloc[:ne] = pos_s[a:b_] - TILE_STARTS[t]
                wv = np.zeros(n_slots, dtype=np.float32)
                wv[:ne] = w_s[a:b_]
                S = np.zeros((n_slots, P), dtype=np.float32)
                valid = dstloc < P
                S[np.nonzero(valid)[0], dstloc[valid]] = wv[valid]
                s_parts[islo].append(S.reshape(-1, P, P))
                idx_parts[islo].append(idx.reshape(-1, 16).T.astype(np.int16))
        pc = {"agg1T": np.ascontiguousarray(agg1T)}
        for islo, nm in ((True, "lo"), (False, "hi")):
            if idx_parts[islo]:
                pc[f"idx_{nm}"] = np.ascontiguousarray(
                    np.tile(np.concatenate(idx_parts[islo], axis=1), (8, 1)))
                # store S partition-major ([P, c, n]) so the group DMA
                # reads each partition contiguously (no strided rearrange)
                pc[f"s_{nm}"] = np.ascontiguousarray(
                    np.concatenate(s_parts[islo], axis=0).transpose(1, 0, 2))
            else:
                pc[f"idx_{nm}"] = np.zeros((128, 1), dtype=np.int16)
                pc[f"s_{nm}"] = np.zeros((P, 1, P), dtype=np.float32)
        per_core.append(pc)
    return sched_lo, sched_hi, per_core


# ----------------------------------------------------------------------------
# Bass program builder (depends only on sched_lo / sched_hi)
# ----------------------------------------------------------------------------

def _build(sched_lo, sched_hi):
    nc = bacc.Bacc("TRN2")
    ch_lo = np.ceil(sched_lo / P).astype(np.int64)
    ch_hi = np.ceil(sched_hi / P).astype(np.int64)
    idx_lo_cols = int(sched_lo.sum()) // 16
    idx_hi_cols = max(int(sched_hi.sum()) // 16, 1)
    tot_clo = int(ch_lo.sum())
    tot_chi = max(int(ch_hi.sum()), 1)
    offi_lo = np.concatenate([[0], np.cumsum(sched_lo // 16)]).astype(int)
    offi_hi = np.concatenate([[0], np.cumsum(sched_hi // 16)]).astype(int)
    offc_lo = np.concatenate([[0], np.cumsum(ch_lo)]).astype(int)
    offc_hi = np.concatenate([[0], np.cumsum(ch_hi)]).astype(int)

    agg1T_d = nc.declare_dram_parameter("agg1T", [384, NPC], DT, isOutput=False)
    Ws, bs = [], []
    for i in range(5):
        fi, fo = DIMS[i], DIMS[i + 1]
        Ws.append(nc.declare_dram_parameter(f"W{i+1}", [fi, fo], DT, isOutput=False))
        bs.append(nc.declare_dram_parameter(f"b{i+1}", [fo, 1], F32, isOutput=False))
    b4r_d = nc.declare_dram_parameter("b4r", [1, 128], DT, isOutput=False)
    b5r_d = nc.declare_dram_parameter("b5r", [1, 2048], DT, isOutput=False)
    idx_lo_d = nc.declare_dram_parameter("idx_lo", [128, idx_lo_cols], I16, isOutput=False)
    idx_hi_d = nc.declare_dram_parameter("idx_hi", [128, idx_hi_cols], I16, isOutput=False)
    s_lo_d = nc.declare_dram_parameter("s_lo", [P, tot_clo, P], DT, isOutput=False)
    s_hi_d = nc.declare_dram_parameter("s_hi", [P, tot_chi, P], DT, isOutput=False)
    out_d = nc.declare_dram_parameter("out", [NPC, 2048], DT, isOutput=True)

    with tile.TileContext(nc) as tc:
        with (
            tc.tile_pool(name="dram", bufs=1, space="DRAM") as dram,
            tc.tile_pool(name="cpool", bufs=1) as cpool,
            tc.tile_pool(name="sb", bufs=2) as sb,
            tc.tile_pool(name="pagg", bufs=1, space="PSUM") as pagg,
            tc.tile_pool(name="pmm", bufs=4, space="PSUM") as pmm,
        ):
            # ---- internal DRAM ----
            g2_d = dram.tile([NPC, 512], DT)
            g3_d = dram.tile([NPC, 256], DT)
            g4_d = dram.tile([NPC, 128], DT)
            h4_d = dram.tile([NPC, 128], DT)
            # split tables: A = nodes [0, 25000), B = nodes [25000, 50000)
            T2a = dram.tile([HALF, 512], DT, addr_space="Shared")
            T2b = dram.tile([HALF, 512], DT, addr_space="Shared")
            T3a = dram.tile([HALF, 256], DT, addr_space="Shared")
            T3b = dram.tile([HALF, 256], DT, addr_space="Shared")
            T4a = dram.tile([HALF, 128], DT, addr_space="Shared")
            T4b = dram.tile([HALF, 128], DT, addr_space="Shared")
            T5a = dram.tile([HALF, 128], DT, addr_space="Shared")
            T5b = dram.tile([HALF, 128], DT, addr_space="Shared")

            # ---- resident SBUF ----
            ones_sb = cpool.tile([1, 128], DT, name="ones")
            nc.any.memset(ones_sb[:], 1.0)
            b4r_sb = cpool.tile([1, 128], DT, name="b4rsb")
            nc.sync.dma_start(b4r_sb[:], b4r_d[:])
            b5r_sb = cpool.tile([1, 2048], DT, name="b5rsb")
            nc.sync.dma_start(b5r_sb[:], b5r_d[:])
            idx_lo_sb = cpool.tile([128, idx_lo_cols], I16, name="idxlo")
            nc.sync.dma_start(idx_lo_sb[:], idx_lo_d[:])
            idx_hi_sb = cpool.tile([128, idx_hi_cols], I16, name="idxhi")
            nc.sync.dma_start(idx_hi_sb[:], idx_hi_d[:])

            rg = [list(range(NC))]

            def load_w(i):
                fi, fo = DIMS[i], DIMS[i + 1]
                ks = []
                for k in range(_ceil_div(fi, P)):
                    kk = min(P, fi - k * P)
                    t_ = cpool.tile([P, fo], DT, name=f"w{i}_{k}", tag=f"w{i}k{k}")
                    nc.sync.dma_start(t_[:kk, :], Ws[i][k * P : k * P + kk, :])
                    ks.append((t_, kk))
                return ks

            def load_bcol(i):
                fo = DIMS[i + 1]
                nchunk = _ceil_div(fo, P)
                t_ = cpool.tile([P, 16], F32, name=f"bc{i}", tag=f"bcol{i}")
                for m in range(nchunk):
                    mm = min(P, fo - m * P)
                    nc.sync.dma_start(t_[:mm, m : m + 1], bs[i][m * P : m * P + mm, :])
                return t_

            def allgather2(src_d, dst_a, dst_b):
                nc.gpsimd.collective_compute(
                    "AllGather", mybir.AluOpType.bypass, replica_groups=rg,
                    ins=[src_d[:HPC, :].opt()], outs=[dst_a[:].opt()],
                )
                nc.gpsimd.collective_compute(
                    "AllGather", mybir.AluOpType.bypass, replica_groups=rg,
                    ins=[src_d[HPC:, :].opt()], outs=[dst_b[:].opt()],
                )

            # ================= aggregation =================
            def aggregate(layer, tab_a, tab_b, out_cb, node_major=False):
                """Gather + aggregate all dst tiles.

                layer 0 reads host-shipped pre-gathered x rows via plain DMA;
                other layers dma_gather rows from tab_a (src<25000) / tab_b.
                16-granular schedules, partial-K matmuls on last chunks.
                out_cb(t, tw, pts): per-F-chunk PSUM banks for dst tile t.
                """
                fa = LAYER_FA[layer]
                fap = LAYER_FA_PAD[layer]
                nfc = _ceil_div(fa, P)
                LAG = 0 if layer == 0 else 2

                def emit_front(g0):
                    tiles = list(range(g0, min(g0 + 4, N_TILES)))
                    t0, t1 = tiles[0], tiles[-1]
                    clo_g = int(offc_lo[t1 + 1] - offc_lo[t0])
                    g_chi = int(offc_hi[t1 + 1] - offc_hi[t0])
                    slo_sb = sb.tile([128, clo_g, P], DT, bufs=3,
                                     name=f"slo_{layer}_{g0}", tag="slo")
                    nc.sync.dma_start(
                        slo_sb[:], s_lo_d[:, offc_lo[t0] : offc_lo[t1 + 1], :])
                    shi_sb = None
                    if g_chi > 0:
                        shi_sb = sb.tile([128, g_chi, P], DT, name=f"shi_{layer}_{g0}", tag="shi")
                        nc.sync.dma_start(
                            shi_sb[:], s_hi_d[:, offc_hi[t0] : offc_hi[t1 + 1], :])
                    if True:
                        # batched lo gathers (Q7 fixed cost is per call);
                        # scheds are 128-granular so tile chunks align
                        hg_lo_g = sb.tile([128, clo_g, fap], DT, bufs=4,
                                          name=f"hglo_{layer}_{g0}", tag="hglo")
                        for ta, tb in _pack_calls(tiles, sched_lo):
                            ni = int(sched_lo[ta : tb + 1].sum())
                            c_a = int(offc_lo[ta] - offc_lo[t0])
                            c_b = int(offc_lo[tb + 1] - offc_lo[t0])
                            nc.gpsimd.dma_gather(
                                hg_lo_g[:, c_a:c_b, :], tab_a,
                                idx_lo_sb[:, offi_lo[ta] : offi_lo[tb + 1]],
                                ni, ni, fap,
                            )
                        hg_hi_g = "defer"
                    return (tiles, t0, t1, g_chi, slo_sb, shi_sb, hg_lo_g, hg_hi_g)

                def emit_back(st):
                    tiles, t0, t1, g_chi, slo_sb, shi_sb, hg_lo_g, hg_hi_g = st
                    if hg_hi_g == "defer":
                        # deferred hi gathers (wait on the late AG half)
                        hg_hi_g = None
                        if int(sched_hi[t0 : t1 + 1].sum()) > 0:
                            hg_hi_g = sb.tile([128, g_chi, fap], DT,
                                              name=f"hghi_{layer}_{t0}", tag="hghi")
                            for ta, tb in _pack_calls(tiles, sched_hi):
                                ni = int(sched_hi[ta : tb + 1].sum())
                                if ni == 0:
                                    continue
                                c_a = int(offc_hi[ta] - offc_hi[t0])
                                c_b = int(offc_hi[tb + 1] - offc_hi[t0])
                                nc.gpsimd.dma_gather(
                                    hg_hi_g[:, c_a:c_b, :], tab_b,
                                    idx_hi_sb[:, offi_hi[ta] : offi_hi[tb + 1]],
                                    ni, ni, fap,
                                )
                    do_tiles(tiles, t0, slo_sb, shi_sb, hg_lo_g, hg_hi_g)

                def do_tiles(tiles, t0, slo_sb, shi_sb, hg_lo_g, hg_hi_g):
                    for t in tiles:
                        tw = TILE_WIDTHS[t]
                        chunks = []
                        for ci in range(int(ch_lo[t])):
                            c = int(offc_lo[t] - offc_lo[t0]) + ci
                            chunks.append((hg_lo_g, c, slo_sb, c, P))
                        for ci in range(int(ch_hi[t])):
                            c = int(offc_hi[t] - offc_hi[t0]) + ci
                            chunks.append((hg_hi_g, c, shi_sb, c, P))
                        # one PSUM bank per accumulation group (first_mm's
                        # has_written clear is partition-row x bank granular)
                        pts = [
                            pagg.tile([P, P], F32, name=f"pt_{layer}_{t}_{fc}",
                                      tag=f"pagg{fc}", space="PSUM", bufs=1)
                            for fc in range(nfc)
                        ]
                        nch = len(chunks)
                        if node_major:
                            for ci, (hg, hc, ssb, sc, K) in enumerate(chunks):
                                nc.tensor.matmul(
                                    pts[0][:, :fa], ssb[:K, sc, :], hg[:K, hc, :fa],
                                    start=(ci == 0), stop=False,
                                )
                            nc.tensor.matmul(  # += bias row
                                pts[0][:, :fa], ones_sb[:1, :], b4r_sb[:1, :fa],
                                start=False, stop=True,
                            )
                        else:
                            for ci, (hg, hc, ssb, sc, K) in enumerate(chunks):
                                for fc in range(nfc):
                                    fw = min(P, fa - fc * P)
                                    nc.tensor.matmul(
                                        pts[fc][:fw, :],
                                        hg[:K, hc, fc * P : fc * P + fw],
                                        ssb[:K, sc, :],
                                        start=(ci == 0), stop=(ci == nch - 1),
                                    )
                        out_cb(t, tw, pts)

                pending = []
                for g0 in range(0, N_TILES, 4):
                    pending.append(emit_front(g0))
                    if len(pending) > LAG:
                        emit_back(pending.pop(0))
                while pending:
                    emit_back(pending.pop(0))

            # ============ fused window pipeline ============
            # Per 4-tile group (<=512 contiguous local nodes): agg psums are
            # evicted into F-major stage tiles (SBUF), the NEXT layer's dense
            # consumes them directly (activations never round-trip DRAM), the
            # dense output g window is written to DRAM, and the AllGather
            # halves fire as soon as their input rows are complete.
            w1 = load_w(0)
            b1c = load_bcol(0)
            w2 = load_w(1)
            b2c = load_bcol(1)
            w3 = load_w(2)
            b3c = load_bcol(2)
            w4 = load_w(3)
            w5 = load_w(4)

            def make_ag(src_d, dst_a, dst_b):
                def ag(which):
                    if which == 0:
                        nc.gpsimd.collective_compute(
                            "AllGather", mybir.AluOpType.bypass, replica_groups=rg,
                            ins=[src_d[:HPC, :].opt()], outs=[dst_a[:].opt()],
                        )
                    else:
                        nc.gpsimd.collective_compute(
                            "AllGather", mybir.AluOpType.bypass, replica_groups=rg,
                            ins=[src_d[HPC:, :].opt()], outs=[dst_b[:].opt()],
                        )
                return ag

            def make_dense_window(li, w_tiles, nk, fo, g_dst):
                # node-major dense from F-major stage chunks (lhsT = stages)
                def dense_fn(c0, cols, stages):
                    for m0 in range(0, cols, P):
                        mw = min(P, cols - m0)
                        pm = pmm.tile([P, 512], F32, name=f"pm_{li}_{c0}_{m0}",
                                      tag="pmm", space="PSUM")
                        for k in range(nk):
                            kk = w_tiles[k][1]
                            nc.tensor.matmul(
                                pm[:mw, :fo],
                                stages[k][:kk, m0 : m0 + mw],
                                w_tiles[k][0][:kk, :fo],
                                start=(k == 0), stop=(k == nk - 1),
                            )
                        ev = sb.tile([P, 512], DT, name=f"ev_{li}_{c0}_{m0}", tag="ev")
                        nc.vector.tensor_copy(ev[:mw, :fo], pm[:mw, :fo])
                        nc.sync.dma_start(
                            g_dst[c0 + m0 : c0 + m0 + mw, :fo], ev[:mw, :fo])
                return dense_fn

            def l1_window(c0, cols, stages0):
                # L1 dense (F-major h1 chunks, lrelu+b1) then L2 dense -> g2
                h1st = [sb.tile([P, 512], DT, name=f"h1_{c0}_{m}", tag=f"h1st{m}")
                        for m in range(8)]
                for m in range(8):
                    pm = pmm.tile([P, 512], F32, name=f"apm_{c0}_{m}",
                                  tag="pmm", space="PSUM")
                    for k in range(3):
                        kk = (128, 128, 44)[k]
                        nc.tensor.matmul(
                            pm[:, :cols],
                            w1[k][0][:kk, m * P : (m + 1) * P],
                            stages0[k][:kk, :cols],
                            start=(k == 0), stop=(k == 2),
                        )
                    nc.scalar.activation(
                        h1st[m][:, :cols], pm[:, :cols], LRELU,
                        bias=b1c[:, m : m + 1], alpha=NEG_SLOPE,
                    )
                make_dense_window(2, w2, 8, 512, g2_d)(c0, cols, h1st)

            def out_window(c0, cols, stages):
                for m0 in range(0, cols, P):
                    mw = min(P, cols - m0)
                    ev = sb.tile([P, 2048], DT, name=f"oev_{c0}_{m0}", tag="oev")
                    for n in range(4):
                        pm = pmm.tile([P, 512], F32, name=f"pm5_{c0}_{m0}_{n}",
                                      tag="pmm", space="PSUM")
                        nc.tensor.matmul(
                            pm[:mw, :], stages[0][:, m0 : m0 + mw],
                            w5[0][0][:, n * 512 : (n + 1) * 512],
                            start=True, stop=False,
                        )
                        nc.tensor.matmul(  # += bias row (K=1 outer product)
                            pm[:mw, :], ones_sb[:1, :mw],
                            b5r_sb[:1, n * 512 : (n + 1) * 512],
                            start=False, stop=True,
                        )
                        nc.vector.tensor_copy(
                            ev[:mw, n * 512 : (n + 1) * 512], pm[:mw, :])
                    nc.sync.dma_start(out_d[c0 + m0 : c0 + m0 + mw, :], ev[:mw, :])

            def make_fused_out(layer, nfc, fa, bias_col, lrelu, window_fn, ag_fn):
                state = {"stages": None, "c0": 0, "col": 0, "ag_a": False}

                def cb(t, tw, pts):
                    if t % 4 == 0:
                        state["stages"] = [
                            sb.tile([P, 512], DT, name=f"st_{layer}_{t}_{fc}", tag=f"st{fc}")
                            for fc in range(nfc)
                        ]
                        state["c0"] = TILE_STARTS[t]
                        state["col"] = 0
                    col = state["col"]
                    for fc in range(nfc):
                        fw = min(P, fa - fc * P)
                        if lrelu:
                            nc.scalar.activation(
                                state["stages"][fc][:fw, col : col + tw],
                                pts[fc][:fw, :tw],
                                LRELU, bias=bias_col[:, fc : fc + 1], alpha=NEG_SLOPE,
                            )
                        else:
                            nc.vector.tensor_copy(
                                state["stages"][fc][:fw, col : col + tw],
                                pts[fc][:fw, :tw],
                            )
                    state["col"] = col + tw
                    if t % 4 == 3 or t == N_TILES - 1:
                        window_fn(state["c0"], state["col"], state["stages"])
                        if ag_fn is not None:
                            covered = state["c0"] + state["col"]
                            if not state["ag_a"] and covered >= HPC:
                                ag_fn(0)
                                state["ag_a"] = True
                            if t == N_TILES - 1:
                                ag_fn(1)

                return cb

            # ================= the network =================
            # L1: host-aggregated x (agg1T) -> [L1 dense -> L2 dense] -> g2
            ag2 = make_ag(g2_d, T2a, T2b)
            ag2a_done = False
            for g0 in range(0, N_TILES, 4):
                tiles0 = list(range(g0, min(g0 + 4, N_TILES)))
                c0 = TILE_STARTS[tiles0[0]]
                cols = sum(TILE_WIDTHS[t] for t in tiles0)
                stages0 = [
                    sb.tile([P, 512], DT, name=f"a1_{g0}_{fc}", tag=f"st{fc}")
                    for fc in range(3)
                ]
                for fc in range(3):
                    fw = (128, 128, 44)[fc]
                    nc.sync.dma_start(
                        stages0[fc][:fw, :cols],
                        agg1T_d[fc * P : fc * P + fw, c0 : c0 + cols],
                    )
                l1_window(c0, cols, stages0)
                if not ag2a_done and c0 + cols >= HPC:
                    ag2(0)
                    ag2a_done = True
                if tiles0[-1] == N_TILES - 1:
                    ag2(1)

            # L2: aggregate g2 (Lrelu+b2) -> L3 dense -> g3, AG3
            aggregate(1, T2a[:, :], T2b[:, :],
                      make_fused_out(1, 4, 512, b2c, True,
                                     make_dense_window(3, w3, 4, 256, g3_d),
                                     make_ag(g3_d, T3a, T3b)))

            # L3: aggregate g3 (Lrelu+b3) -> L4 dense -> g4, AG4
            aggregate(2, T3a[:, :], T3b[:, :],
                      make_fused_out(2, 2, 256, b3c, True,
                                     make_dense_window(4, w4, 2, 128, g4_d),
                                     make_ag(g4_d, T4a, T4b)))

            # L4: aggregate g4 node-major (+b4 via matmul, Lrelu) -> h4, AG5
            ag5 = make_ag(h4_d, T5a, T5b)
            l4_state = {"ag_a": False}

            def l4_out(t, tw, pts):
                ev = sb.tile([P, 512], DT, name=f"l4ev_{t}", tag="ev")
                nc.scalar.activation(ev[:tw, :128], pts[0][:tw, :128], LRELU, alpha=NEG_SLOPE)
                nc.scalar.dma_start(
                    h4_d[TILE_STARTS[t] : TILE_STARTS[t] + tw, :], ev[:tw, :128])
                covered = TILE_STARTS[t] + tw
                if not l4_state["ag_a"] and covered >= HPC:
                    ag5(0)
                    l4_state["ag_a"] = True
                if t == N_TILES - 1:
                    ag5(1)

            aggregate(3, T4a[:, :], T4b[:, :], l4_out, node_major=True)

            # L5: aggregate h4 -> out dense (W5 + b5) per window -> out
            aggregate(4, T5a[:, :], T5b[:, :],
                      make_fused_out(4, 1, 128, None, False, out_window, None))

    nc.compile()
    return nc


# ----------------------------------------------------------------------------
# Entry point
# ----------------------------------------------------------------------------

_CACHE = {}


def _run(inputs, trace=False):
    x = np.asarray(inputs["x"], dtype=np.float32)
    edge_index = np.asarray(inputs["edge_index"])
    sched_lo, sched_hi, per_core = _prep(edge_index, x)

    key = (tuple(sched_lo.tolist()), tuple(sched_hi.tolist()))
    if key not in _CACHE:
        _CACHE[key] = _build(sched_lo, sched_hi)
    nc = _CACHE[key]

    common = {}
    for i in range(5):
        common[f"W{i+1}"] = np.ascontiguousarray(
            np.asarray(inputs[f"W{i+1}"], dtype=np.float32).astype(NPDT))
        common[f"b{i+1}"] = np.ascontiguousarray(
            np.asarray(inputs[f"b{i+1}"], dtype=np.float32).reshape(-1, 1))
    common["b4r"] = np.ascontiguousarray(common["b4"].reshape(1, 128).astype(NPDT))
    common["b5r"] = np.ascontiguousarray(
        np.asarray(inputs["b5"], dtype=np.float32).reshape(1, 2048).astype(NPDT))

    in_maps = [
        {**common, **{k: (v.astype(NPDT) if k.startswith("s_") else v)
                      for k, v in per_core[c].items()}}
        for c in range(NC)
    ]
    res = run_bass_kernel_spmd(nc, in_maps, core_ids=list(range(NC)), trace=trace)
    # reassemble: core c rows [0:HPC] -> global [c*HPC:(c+1)*HPC],
    #             rows [HPC:NPC] -> global [HALF + c*HPC : HALF + (c+1)*HPC]
    out = np.empty((N_NODES, 2048), dtype=np.float32)
    for c in range(NC):
        oc = np.asarray(res.results[c]["out"], dtype=np.float32)
        out[c * HPC : (c + 1) * HPC] = oc[:HPC]
        out[HALF + c * HPC : HALF + (c + 1) * HPC] = oc[HPC:]
    return out, res


def kernel(**inputs):
    out, _ = _run(inputs, trace=False)
    return out

